# revision 16
# baseline (speedup 1.0000x reference)
"""Criss-cross (CCNet) attention kernel for Trainium2, 8 NeuronCores.

Sharding: core c in 0..7 -> batch b = c//2, value-channel half h = c%2.
Each core computes, for its (b, h), the full joint row+column softmax
attention over 256 of the 512 value/output channels.

Design (v3, contiguous column pass):
  - All matmuls are fp16 (1 cyc/row at any moving size, vs fp32r's 4x
    penalty under 256), with fp32 PSUM accumulation.
  - Energies E = k.q are shifted by a global constant DELTA before exp;
    P = exp(E-DELTA) is stored in bf16 (wide exponent range covers the
    ~33-nat spread of per-row maxima; fp16 cannot).  The shift cancels
    exactly in the final combine since both passes share it.
  - Outputs are UN-normalized:  orow = P^T V  in bf16 (wide exponent
    range), plus the row/col sums S (fp32).  The host computes
    out = (o_row + o_col^T) / (S_row + S_col), which equals the exact
    softmax combine; no on-device division or scaling at all.
  - V is staged through DRAM x-major as vscr[x][j][c] (one 512B-
    descriptor write per row chunk, one strided 512B-descriptor gather
    per column chunk); the partition transpose rides the DRAM
    addressing, never a slow single-partition SBUF DMA.
  - The column pass operands are x-major SBUF copies QT/K2T written
    during the row pass (scalar evacuates QT from PSUM with a strided
    AP; DVE free-dim-transposes K2 into K2T), so every column-pass
    matmul streams contiguous SBUF exactly like the row pass.  This
    keeps the PE at its full 2.4 GHz p-state instead of the 1.2 GHz
    mid state it falls to after every stall.
  - Deep software pipeline in both passes: iteration i runs
    projections(i), energies(i-1), aggregation+evac(i-2), so the
    tensor queue never drains.
"""

import numpy as np

import concourse.tile as tile
from concourse import bacc, mybir
from concourse.bass_utils import run_bass_kernel_spmd

B, C, H, W = 4, 512, 128, 128
CQK = C // 8          # 64
CV = C // 2           # 256 v channels per core
HW = H * W
N_CORES = 8
NCH = 32              # row chunks (4 rows each)
NHG = 32              # col half-groups (4 cols each)
NRING = 8             # VTB ring size in half-groups (32 slots)
DEPTH = 6             # VTB prefetch depth in half-groups
DELTA = 41.0          # exp shift (max energy on this data ~50.7)
VBW = 258             # v slot width: 256 channels + 2 ones columns

F32 = mybir.dt.float32
F16 = mybir.dt.float16
BF16 = mybir.dt.bfloat16
EXP = mybir.ActivationFunctionType.Exp
COPY = mybir.ActivationFunctionType.Copy

_CACHE = {}


def _build(with_bias):
    nc = bacc.Bacc("TRN2", target_bir_lowering=False, debug=False,
                   num_devices=N_CORES)
    nck = 5 if with_bias else 4    # contraction chunks (last = 2 bias rows)

    xin = nc.dram_tensor("xin", [NCH, 128, 2048], F16,
                         kind="ExternalInput").ap()
    xbias = nc.dram_tensor("xbias", [NCH, 2, 2048], F16,
                           kind="ExternalInput").ap() if with_bias else None
    wqk = nc.dram_tensor("wqk", [C + (2 if with_bias else 0), 128], F16,
                         kind="ExternalInput").ap()
    wv = nc.dram_tensor("wv", [C + (2 if with_bias else 0), CV], F16,
                        kind="ExternalInput").ap()
    negid = nc.dram_tensor("negid", [128, 128], F16,
                           kind="ExternalInput").ap()
    id4 = nc.dram_tensor("id4", [128, 512], F16, kind="ExternalInput").ap()

    # orow laid out [x, y, 258]; ocol laid out [y, x, 258]; channel 256 of
    # each carries the (unnormalized) softmax partition sum in bf16
    orow = nc.dram_tensor("orow", [W, H, VBW], BF16,
                          kind="ExternalOutput").ap()
    ocol = nc.dram_tensor("ocol", [H, W, VBW], BF16,
                          kind="ExternalOutput").ap()


    with tile.TileContext(nc) as tc:
        with (
            tc.tile_pool(name="cst", bufs=1) as cst,
            tc.tile_pool(name="dram", bufs=1, space="DRAM") as dramp,
            tc.tile_pool(name="xs", bufs=3) as xsp,
            tc.tile_pool(name="p4", bufs=3) as p4p,
            tc.tile_pool(name="p4c", bufs=6) as p4cp,
            tc.tile_pool(name="o16r", bufs=2) as o16rp,
            tc.tile_pool(name="o16c", bufs=4) as o16cp,
            tc.tile_pool(name="psbig", bufs=2, space="PSUM") as psbigp,
            tc.tile_pool(name="psv", bufs=2, space="PSUM") as psvp,
            tc.tile_pool(name="psO", bufs=2, space="PSUM") as psOp,
        ):
            # ---- startup: critical path first (xin0, WQK), one DMA each ----
            xpre = {}
            xt0 = xsp.tile([128, 2048], F16, tag="xs", name="xpre0")
            nc.sync.dma_start(xt0[:], xin[0])
            WQK = cst.tile([128, nck * 128], F16)
            nc.sync.dma_start(
                WQK[:].rearrange("p (k c) -> p k c", c=128)[:, 0:4, :],
                wqk[0:512, :].rearrange("(k p) c -> p k c", p=128))
            xt1 = xsp.tile([128, 2048], F16, tag="xs", name="xpre1")
            nc.sync.dma_start(xt1[:], xin[1])
            WV = cst.tile([128, nck * CV], F16)
            nc.sync.dma_start(
                WV[:].rearrange("p (k c) -> p k c", c=CV)[:, 0:4, :],
                wv[0:512, :].rearrange("(k p) c -> p k c", p=128))
            if with_bias:
                nc.sync.dma_start(WQK[0:2, 4 * 128:5 * 128], wqk[512:514, :])
                nc.sync.dma_start(WV[0:2, 4 * CV:5 * CV], wv[512:514, :])
                for ch0 in range(2):
                    xbt = xsp.tile([2, 2048], F16, tag="xb",
                                   name=f"xbpre{ch0}")
                    nc.sync.dma_start(xbt[:], xbias[ch0])
                    xpre[ch0] = ((xt0, xt1)[ch0], xbt)
            else:
                xpre[0] = (xt0, None)
                xpre[1] = (xt1, None)
            NEGID = cst.tile([128, 128], F16)
            nc.sync.dma_start(NEGID[:], negid[:])
            ID4 = cst.tile([128, 512], F16)
            nc.sync.dma_start(ID4[:], id4[:])

            QK = cst.tile([128, HW], F16)     # y-major [c, (y, x)]
            K2 = cst.tile([64, HW], F16)      # k y-major on partitions 0-63
            QT = cst.tile([64, HW], F16)      # q x-major [c, (x, y)]
            K2T = cst.tile([64, HW], F16)     # k x-major [c, (x, y)]
            VTB = cst.tile([128, NRING * 4 * VBW], BF16)
            VB = cst.tile([128, 24 * VBW], BF16)
            BIASC = cst.tile([128, 1], F32)
            nc.vector.memset(BIASC[:], -DELTA)
            # v staging scratch as a DRAM-pool tile: the tile framework
            # orders the chunk-write DMAs against the gather DMAs (raw
            # dram_tensor accesses get NO DMA-vs-DMA ordering at all)
            VSCR = dramp.tile([W, H, CV], BF16)
            vtb_view = VTB[:].rearrange("p (s w) -> p s w", w=VBW)
            vb_view = VB[:].rearrange("p (s w) -> p s w", w=VBW)
            nc.vector.memset(vtb_view[:, :, 256:258], 1.0)
            nc.vector.memset(vb_view[:, :, 256:258], 1.0)

            # x-major views [c, x, y] of the x-major tiles
            qt_xm = QT[:].rearrange("c (x y) -> c x y", y=128)
            k2t_xm = K2T[:].rearrange("c (x y) -> c x y", y=128)
            # y-major [c, x, y] views of the row-major tiles (x stride 1,
            # y stride 128) used as transpose sources
            k2_as_xy = K2[:].rearrange("c (y x) -> c x y", x=128)

            # =================== row pass ===================
            state = {}
            state[("xs", 0)] = xpre[0]
            state[("xs", 1)] = xpre[1]

            def load_x(ch):
                xt = xsp.tile([128, 2048], F16, tag="xs")
                nc.sync.dma_start(xt[:], xin[ch])
                xbt = None
                if with_bias:
                    xbt = xsp.tile([2, 2048], F16, tag="xb")
                    nc.sync.dma_start(xbt[:], xbias[ch])
                state[("xs", ch)] = (xt, xbt)

            def mm_in(xs, k, sl):
                xt, xbt = xs
                if k < 4:
                    return xt[:, k * 512:k * 512 + 512][:, sl]
                return xbt[:, sl]

            def row_head(i):
                xs = state.pop(("xs", i))
                csl = slice(i * 512, (i + 1) * 512)
                pqk = psbigp.tile([128, 512], F32, tag="psbig")
                for k in range(nck):
                    rows = 128 if k < 4 else 2
                    nc.tensor.matmul(pqk[:],
                                     WQK[0:rows, k * 128:(k + 1) * 128],
                                     mm_in(xs, k, slice(0, 512)),
                                     start=(k == 0), stop=(k == nck - 1))
                nc.scalar.activation(QK[:, csl], pqk[:], COPY)
                # second evac: q rows into the x-major QT with a strided
                # destination AP (dst (x, y), src (y, x))
                nc.scalar.activation(
                    qt_xm[:, :, i * 4:(i + 1) * 4],
                    pqk[0:64, :].rearrange("c (y x) -> c x y", x=128),
                    COPY)
                nc.gpsimd.tensor_copy(K2[:, csl], QK[64:128, csl])
                idx = i % 6
                for hv in range(2):       # two [128,512] pv tiles: yy pairs
                    pv = psvp.tile([128, 512], F32, tag="psv")
                    for q2 in range(2):
                        yy = hv * 2 + q2
                        xsl = slice(yy * 128, (yy + 1) * 128)
                        for k in range(nck):
                            rows = 128 if k < 4 else 2
                            nc.tensor.matmul(
                                pv[:, q2 * 256:(q2 + 1) * 256],
                                mm_in(xs, k, xsl),
                                WV[0:rows, k * CV:(k + 1) * CV],
                                start=(k == 0), stop=(k == nck - 1))
                    # evac both yy slots in one strided DVE copy
                    nc.vector.tensor_copy(
                        vb_view[:, idx * 4 + hv * 2:idx * 4 + hv * 2 + 2,
                                0:256],
                        pv[:].rearrange("p (s w) -> p s w", w=256))

            def row_mid(j):
                pE = psbigp.tile([128, 512], F32, tag="psbig")
                for yy in range(4):
                    y = j * 4 + yy
                    ysl = slice(y * 128, (y + 1) * 128)
                    nc.tensor.matmul(pE[:, yy * 128:(yy + 1) * 128],
                                     K2[:, ysl], QK[0:64, ysl],
                                     start=True, stop=True)
                p4 = p4p.tile([128, 512], BF16, tag="p4")
                nc.scalar.activation(p4[:], pE[:], EXP, bias=BIASC[:])
                state[("p4", j)] = p4
                # free-dim transpose of this chunk's k rows into K2T (DVE;
                # K2[:, csl] was written by gpsimd in row_head(j))
                csl = slice(j * 512, (j + 1) * 512)
                nc.vector.tensor_copy(
                    k2t_xm[:, :, j * 4:(j + 1) * 4],
                    k2_as_xy[:, :, j * 4:(j + 1) * 4])

            def row_tail(j):
                idx = j % 6
                p4 = state.pop(("p4", j))
                o16 = o16rp.tile([128, 4 * VBW], BF16, tag="o16r")
                for half in range(2):
                    pO = psOp.tile([128, 1024], F32, tag="psO")
                    for q2 in range(2):
                        yy = half * 2 + q2
                        nc.tensor.matmul(
                            pO[:, q2 * 512:q2 * 512 + VBW],
                            p4[:, yy * 128:(yy + 1) * 128],
                            VB[:, (idx * 4 + yy) * VBW:
                               (idx * 4 + yy + 1) * VBW],
                            start=True, stop=True)
                    for q2 in range(2):
                        yy = half * 2 + q2
                        src = pO[:, q2 * 512:q2 * 512 + VBW]
                        dst = o16[:, yy * VBW:(yy + 1) * VBW]
                        if q2 == 0:
                            nc.scalar.activation(dst, src, COPY)
                        else:
                            nc.vector.tensor_copy(dst, src)
                # orow chunk j: dst [x, 4 y, 256] contiguous per partition
                nc.sync.dma_start(
                    orow[:, j * 4:(j + 1) * 4, :],
                    o16[:].rearrange("p (t c) -> p t c", c=VBW))
                # stage VB slots to DRAM x-major: dst [x, 4 y, c] matches
                # the VB source order (x-part, yy, c); 512B descriptors
                nc.sync.dma_start(
                    VSCR[:, j * 4:(j + 1) * 4, :],
                    vb_view[:, idx * 4:idx * 4 + 4, 0:256])
                # pre-gather the first ring pairs for the column pass in
                # y-slices as soon as each VSCR y-region is complete
                # (y<64 after chunk 15, y 64..95 after chunk 23), riding
                # the row window's DMA headroom instead of stalling the
                # column-pass spin-up
                if j in (17, 19, 21, 23):
                    p = (j - 17) // 2
                    s = (2 * p) % NRING
                    nc.sync.dma_start(
                        vtb_view[0:64, s * 4:s * 4 + 8, 0:256],
                        VSCR[p * 8:(p + 1) * 8, 0:64, :].rearrange(
                            "t j c -> j t c"))
                if j in (25, 27, 29, 31):
                    p = (j - 25) // 2
                    s = (2 * p) % NRING
                    nc.sync.dma_start(
                        vtb_view[64:96, s * 4:s * 4 + 8, 0:256],
                        VSCR[p * 8:(p + 1) * 8, 64:96, :].rearrange(
                            "t j c -> j t c"))

            for i in range(NCH + 2):
                if i < NCH:
                    if i + 2 < NCH:
                        load_x(i + 2)
                    row_head(i)
                if 1 <= i < NCH + 1:
                    row_mid(i - 1)
                if i >= 2:
                    row_tail(i - 2)

            # =================== column pass ===================
            # Mirrors the row pass exactly with x <-> y swapped; all
            # operands contiguous (QT/K2T x-major, VTB gathered x-major).
            cstate = {}

            def vtb_fetch_pair(p):
                # gather the 8 column slots of half-group pair p = (2p, 2p+1)
                # in ONE DMA: the sync engine's ~650ns per-DMA descriptor
                # generation is the column pass's scarcest resource
                s = (2 * p) % (NRING)
                nc.sync.dma_start(
                    vtb_view[:, s * 4:s * 4 + 8, 0:256],
                    VSCR[p * 8:(p + 1) * 8, :, :].rearrange(
                        "t j c -> j t c"))

            def col_mid(i):
                # alternate pE between the psbig and psv pools (psv is idle
                # in the column pass): 4 physical banks deep, so E(i+1)
                # depends on exp(i-1) instead of exp(i) -- keeps the scalar
                # exp latency off the tensor engine's critical loop
                if i % 2 == 0:
                    pE = psbigp.tile([128, 512], F32, tag="psbig")
                else:
                    pE = psvp.tile([128, 512], F32, tag="psv")
                for xx in range(4):
                    x = i * 4 + xx
                    xsl = slice(x * 128, (x + 1) * 128)
                    nc.tensor.matmul(pE[:, xx * 128:(xx + 1) * 128],
                                     K2T[:, xsl], QT[:, xsl],
                                     start=(xx == 0), stop=False)
                nc.tensor.matmul(pE[:], NEGID[:], ID4[:],
                                 start=False, stop=True)
                p4 = p4cp.tile([128, 512], BF16, tag="p4c")
                nc.scalar.activation(p4[:], pE[:], EXP, bias=BIASC[:])
                cstate[i] = p4

            def col_tail(i):
                p4 = cstate.pop(i)
                if i % 2 == 0:
                    cstate["o16"] = o16cp.tile([128, 8 * VBW], BF16,
                                               tag="o16c", name="o16pair")
                o16 = cstate["o16"]
                for h2 in range(2):
                    pO = psOp.tile([128, 1024], F32, tag="psO")
                    for q2 in range(2):
                        xx = h2 * 2 + q2
                        slot = (i % NRING) * 4 + xx
                        nc.tensor.matmul(
                            pO[:, q2 * 512:q2 * 512 + VBW],
                            p4[:, xx * 128:(xx + 1) * 128],
                            VTB[:, slot * VBW:(slot + 1) * VBW],
                            start=True, stop=True)
                    # one batched pair-evacuation per psO tile; keep the
                    # scalar queue nearly exp-only so exp dispatch never
                    # queues behind evac waits: DVE takes pair 0, pair 1
                    # alternates scalar/DVE by iteration parity
                    src = pO[:].rearrange("p (s w) -> p s w", w=512)[
                        :, :, 0:VBW]
                    dst = o16[:].rearrange("p (s w) -> p s w", w=VBW)[
                        :, (i % 2) * 4 + h2 * 2:(i % 2) * 4 + h2 * 2 + 2, :]
                    if h2 == 1 and i % 2 == 0:
                        nc.scalar.activation(dst, src, COPY)
                    else:
                        nc.vector.tensor_copy(dst, src)
                if i % 2 == 1:
                    # one 8-column output DMA per half-group pair
                    nc.sync.dma_start(
                        ocol[:, (i - 1) * 4:(i + 1) * 4, :],
                        o16[:].rearrange("p (t c) -> p t c", c=VBW))
                    # refill this ring pair for pair (i-1)/2 + NRING/2 now
                    # that its last reader (the agg matmuls above) has been
                    # issued -- any earlier would order those reads after
                    # the refill and hand them future data
                    p = (i - 1) // 2
                    if p + NRING // 2 < NHG // 2:
                        vtb_fetch_pair(p + NRING // 2)

            for p in range(NRING // 2):
                # y<96 was pre-gathered during the row pass; only the last
                # y-quarter (complete after the final row chunk) remains
                s = (2 * p) % NRING
                nc.sync.dma_start(
                    vtb_view[96:128, s * 4:s * 4 + 8, 0:256],
                    VSCR[p * 8:(p + 1) * 8, 96:128, :].rearrange(
                        "t j c -> j t c"))
            for i in range(NHG + 2):
                if i < NHG:
                    col_mid(i)
                if i >= 2:
                    col_tail(i - 2)


    nc.compile()
    return nc


def _get_nc(with_bias):
    key = bool(with_bias)
    if key not in _CACHE:
        _CACHE[key] = _build(key)
    return _CACHE[key]


def kernel(x, Wq, bq, Wk, bk, Wv, bv, _trace=False, _raw=False):
    x = np.asarray(x, np.float32)
    Wq = np.asarray(Wq, np.float32)
    Wk = np.asarray(Wk, np.float32)
    Wv = np.asarray(Wv, np.float32)
    bq = np.asarray(bq, np.float32)
    bk = np.asarray(bk, np.float32)
    bv = np.asarray(bv, np.float32)

    with_bias = bool(np.any(bq) or np.any(bk) or np.any(bv))
    nc = _get_nc(with_bias)

    negid_a = (-60000.0 * np.eye(128)).astype(np.float16)
    id4_a = np.tile(np.eye(128), (1, 4)).astype(np.float16)
    wqk_full = np.concatenate([Wq.T, Wk.T], axis=1)       # [C, 128]
    if with_bias:
        bias_qk = np.concatenate([bq, bk])[None, :]
        wqk_full = np.concatenate(
            [wqk_full, bias_qk, np.zeros_like(bias_qk)], axis=0)
    wqk_full = wqk_full.astype(np.float16)

    in_maps = []
    for core in range(N_CORES):
        b, h = core // 2, core % 2
        # xin[ch, p, k*512+w] = x[b, 128k+p, ch*512+w]
        xb = np.ascontiguousarray(
            x[b].reshape(4, 128, NCH, 512).transpose(2, 1, 0, 3)
            .reshape(NCH, 128, 2048)).astype(np.float16)
        wvh = Wv[h * CV:(h + 1) * CV, :].T                # [C, CV]
        if with_bias:
            bvh = bv[h * CV:(h + 1) * CV][None, :]
            wvh = np.concatenate([wvh, bvh, np.zeros_like(bvh)], axis=0)
        m = {
            "xin": xb, "wqk": wqk_full,
            "wv": wvh.astype(np.float16),
            "negid": negid_a, "id4": id4_a,
        }
        if with_bias:
            ob = np.zeros((NCH, 2, 2048), np.float32)
            ob[:, 0, :] = 1.0
            m["xbias"] = ob.astype(np.float16)
        in_maps.append(m)

    res = run_bass_kernel_spmd(nc, in_maps, list(range(N_CORES)),
                               trace=bool(_trace))
    if _raw:
        return res

    out = np.empty((B, C, H, W), np.float32)
    for core in range(N_CORES):
        b, h = core // 2, core % 2
        r = res.results[core]
        o_r3 = r["orow"].astype(np.float32)    # [x, y, 258] unnormalized
        o_c3 = r["ocol"].astype(np.float32)    # [y, x, 258] unnormalized
        g = 1.0 / (o_r3[:, :, 256].T + o_c3[:, :, 256])        # [y, x]
        comb = (o_r3[:, :, 0:256].transpose(1, 0, 2)
                + o_c3[:, :, 0:256]) * g[:, :, None]           # [y, x, c]
        out[b, h * CV:(h + 1) * CV] = comb.transpose(2, 0, 1)

    if _trace:
        return out, res
    return out


# revision 18
# speedup vs baseline: 1.0011x; 1.0011x over previous
"""Criss-cross (CCNet) attention kernel for Trainium2, 8 NeuronCores.

Sharding: core c in 0..7 -> batch b = c//2, value-channel half h = c%2.
Each core computes, for its (b, h), the full joint row+column softmax
attention over 256 of the 512 value/output channels.

Design (v3, contiguous column pass):
  - All matmuls are fp16 (1 cyc/row at any moving size, vs fp32r's 4x
    penalty under 256), with fp32 PSUM accumulation.
  - Energies E = k.q are shifted by a global constant DELTA before exp;
    P = exp(E-DELTA) is stored in bf16 (wide exponent range covers the
    ~33-nat spread of per-row maxima; fp16 cannot).  The shift cancels
    exactly in the final combine since both passes share it.
  - Outputs are UN-normalized:  orow = P^T V  in bf16 (wide exponent
    range), plus the row/col sums S (fp32).  The host computes
    out = (o_row + o_col^T) / (S_row + S_col), which equals the exact
    softmax combine; no on-device division or scaling at all.
  - V is staged through DRAM x-major as vscr[x][j][c] (one 512B-
    descriptor write per row chunk, one strided 512B-descriptor gather
    per column chunk); the partition transpose rides the DRAM
    addressing, never a slow single-partition SBUF DMA.
  - The column pass operands are x-major SBUF copies QT/K2T written
    during the row pass (scalar evacuates QT from PSUM with a strided
    AP; DVE free-dim-transposes K2 into K2T), so every column-pass
    matmul streams contiguous SBUF exactly like the row pass.  This
    keeps the PE at its full 2.4 GHz p-state instead of the 1.2 GHz
    mid state it falls to after every stall.
  - Deep software pipeline in both passes: iteration i runs
    projections(i), energies(i-1), aggregation+evac(i-2), so the
    tensor queue never drains.
"""

import numpy as np

import concourse.tile as tile
from concourse import bacc, mybir
from concourse.bass_utils import run_bass_kernel_spmd

B, C, H, W = 4, 512, 128, 128
CQK = C // 8          # 64
CV = C // 2           # 256 v channels per core
HW = H * W
N_CORES = 8
NCH = 32              # row chunks (4 rows each)
NHG = 32              # col half-groups (4 cols each)
NRING = 8             # VTB ring size in half-groups (32 slots)
DEPTH = 6             # VTB prefetch depth in half-groups
DELTA = 41.0          # exp shift (max energy on this data ~50.7)
VBW = 258             # v slot width: 256 channels + 2 ones columns

F32 = mybir.dt.float32
F16 = mybir.dt.float16
BF16 = mybir.dt.bfloat16
EXP = mybir.ActivationFunctionType.Exp
COPY = mybir.ActivationFunctionType.Copy

_CACHE = {}


def _build(with_bias):
    nc = bacc.Bacc("TRN2", target_bir_lowering=False, debug=False,
                   num_devices=N_CORES)
    nck = 5 if with_bias else 4    # contraction chunks (last = 2 bias rows)

    xin = nc.dram_tensor("xin", [NCH, 128, 2048], F16,
                         kind="ExternalInput").ap()
    xbias = nc.dram_tensor("xbias", [NCH, 2, 2048], F16,
                           kind="ExternalInput").ap() if with_bias else None
    wqk = nc.dram_tensor("wqk", [C + (2 if with_bias else 0), 128], F16,
                         kind="ExternalInput").ap()
    wv = nc.dram_tensor("wv", [C + (2 if with_bias else 0), CV], F16,
                        kind="ExternalInput").ap()
    negid = nc.dram_tensor("negid", [128, 128], F16,
                           kind="ExternalInput").ap()
    id4 = nc.dram_tensor("id4", [128, 512], F16, kind="ExternalInput").ap()

    # orow laid out [x, y, 258]; ocol laid out [y, x, 258]; channel 256 of
    # each carries the (unnormalized) softmax partition sum in bf16
    orow = nc.dram_tensor("orow", [W, H, VBW], BF16,
                          kind="ExternalOutput").ap()
    ocol = nc.dram_tensor("ocol", [H, W, VBW], BF16,
                          kind="ExternalOutput").ap()


    with tile.TileContext(nc) as tc:
        with (
            tc.tile_pool(name="cst", bufs=1) as cst,
            tc.tile_pool(name="dram", bufs=1, space="DRAM") as dramp,
            tc.tile_pool(name="xs", bufs=3) as xsp,
            tc.tile_pool(name="p4", bufs=3) as p4p,
            tc.tile_pool(name="p4c", bufs=6) as p4cp,
            tc.tile_pool(name="o16r", bufs=2) as o16rp,
            tc.tile_pool(name="o16c", bufs=4) as o16cp,
            tc.tile_pool(name="psbig", bufs=2, space="PSUM") as psbigp,
            tc.tile_pool(name="psv", bufs=2, space="PSUM") as psvp,
            tc.tile_pool(name="psO", bufs=2, space="PSUM") as psOp,
        ):
            # ---- startup: critical path first (xin0, WQK), one DMA each ----
            xpre = {}
            xt0 = xsp.tile([128, 2048], F16, tag="xs", name="xpre0")
            nc.sync.dma_start(xt0[:], xin[0])
            WQK = cst.tile([128, nck * 128], F16)
            nc.sync.dma_start(
                WQK[:].rearrange("p (k c) -> p k c", c=128)[:, 0:4, :],
                wqk[0:512, :].rearrange("(k p) c -> p k c", p=128))
            xt1 = xsp.tile([128, 2048], F16, tag="xs", name="xpre1")
            nc.sync.dma_start(xt1[:], xin[1])
            WV = cst.tile([128, nck * CV], F16)
            nc.sync.dma_start(
                WV[:].rearrange("p (k c) -> p k c", c=CV)[:, 0:4, :],
                wv[0:512, :].rearrange("(k p) c -> p k c", p=128))
            if with_bias:
                nc.sync.dma_start(WQK[0:2, 4 * 128:5 * 128], wqk[512:514, :])
                nc.sync.dma_start(WV[0:2, 4 * CV:5 * CV], wv[512:514, :])
                for ch0 in range(2):
                    xbt = xsp.tile([2, 2048], F16, tag="xb",
                                   name=f"xbpre{ch0}")
                    nc.sync.dma_start(xbt[:], xbias[ch0])
                    xpre[ch0] = ((xt0, xt1)[ch0], xbt)
            else:
                xpre[0] = (xt0, None)
                xpre[1] = (xt1, None)
            NEGID = cst.tile([128, 128], F16)
            nc.sync.dma_start(NEGID[:], negid[:])
            ID4 = cst.tile([128, 512], F16)
            nc.sync.dma_start(ID4[:], id4[:])

            QK = cst.tile([128, HW], F16)     # y-major [c, (y, x)]
            K2 = cst.tile([64, HW], F16)      # k y-major on partitions 0-63
            QT = cst.tile([64, HW], F16)      # q x-major [c, (x, y)]
            K2T = cst.tile([64, HW], F16)     # k x-major [c, (x, y)]
            VTB = cst.tile([128, NRING * 4 * VBW], BF16)
            VB = cst.tile([128, 24 * VBW], BF16)
            BIASC = cst.tile([128, 1], F32)
            nc.vector.memset(BIASC[:], -DELTA)
            # v staging scratch as a DRAM-pool tile: the tile framework
            # orders the chunk-write DMAs against the gather DMAs (raw
            # dram_tensor accesses get NO DMA-vs-DMA ordering at all)
            VSCR = dramp.tile([W, H, CV], BF16)
            vtb_view = VTB[:].rearrange("p (s w) -> p s w", w=VBW)
            vb_view = VB[:].rearrange("p (s w) -> p s w", w=VBW)
            nc.vector.memset(vtb_view[:, :, 256:258], 1.0)
            nc.vector.memset(vb_view[:, :, 256:258], 1.0)

            # x-major views [c, x, y] of the x-major tiles
            qt_xm = QT[:].rearrange("c (x y) -> c x y", y=128)
            k2t_xm = K2T[:].rearrange("c (x y) -> c x y", y=128)
            # y-major [c, x, y] views of the row-major tiles (x stride 1,
            # y stride 128) used as transpose sources
            k2_as_xy = K2[:].rearrange("c (y x) -> c x y", x=128)

            # =================== row pass ===================
            state = {}
            state[("xs", 0)] = xpre[0]
            state[("xs", 1)] = xpre[1]

            def load_x(ch):
                xt = xsp.tile([128, 2048], F16, tag="xs")
                nc.sync.dma_start(xt[:], xin[ch])
                xbt = None
                if with_bias:
                    xbt = xsp.tile([2, 2048], F16, tag="xb")
                    nc.sync.dma_start(xbt[:], xbias[ch])
                state[("xs", ch)] = (xt, xbt)

            def mm_in(xs, k, sl):
                xt, xbt = xs
                if k < 4:
                    return xt[:, k * 512:k * 512 + 512][:, sl]
                return xbt[:, sl]

            def row_head(i):
                xs = state.pop(("xs", i))
                csl = slice(i * 512, (i + 1) * 512)
                pqk = psbigp.tile([128, 512], F32, tag="psbig")
                for k in range(nck):
                    rows = 128 if k < 4 else 2
                    nc.tensor.matmul(pqk[:],
                                     WQK[0:rows, k * 128:(k + 1) * 128],
                                     mm_in(xs, k, slice(0, 512)),
                                     start=(k == 0), stop=(k == nck - 1))
                nc.scalar.activation(QK[:, csl], pqk[:], COPY)
                # second evac: q rows into the x-major QT with a strided
                # destination AP (dst (x, y), src (y, x))
                nc.scalar.activation(
                    qt_xm[:, :, i * 4:(i + 1) * 4],
                    pqk[0:64, :].rearrange("c (y x) -> c x y", x=128),
                    COPY)
                nc.gpsimd.tensor_copy(K2[:, csl], QK[64:128, csl])
                idx = i % 6
                for hv in range(2):       # two [128,512] pv tiles: yy pairs
                    pv = psvp.tile([128, 512], F32, tag="psv")
                    for q2 in range(2):
                        yy = hv * 2 + q2
                        xsl = slice(yy * 128, (yy + 1) * 128)
                        for k in range(nck):
                            rows = 128 if k < 4 else 2
                            nc.tensor.matmul(
                                pv[:, q2 * 256:(q2 + 1) * 256],
                                mm_in(xs, k, xsl),
                                WV[0:rows, k * CV:(k + 1) * CV],
                                start=(k == 0), stop=(k == nck - 1))
                    # evac both yy slots in one strided DVE copy
                    nc.vector.tensor_copy(
                        vb_view[:, idx * 4 + hv * 2:idx * 4 + hv * 2 + 2,
                                0:256],
                        pv[:].rearrange("p (s w) -> p s w", w=256))

            def row_mid(j):
                pE = psbigp.tile([128, 512], F32, tag="psbig")
                for yy in range(4):
                    y = j * 4 + yy
                    ysl = slice(y * 128, (y + 1) * 128)
                    nc.tensor.matmul(pE[:, yy * 128:(yy + 1) * 128],
                                     K2[:, ysl], QK[0:64, ysl],
                                     start=True, stop=True)
                p4 = p4p.tile([128, 512], BF16, tag="p4")
                nc.scalar.activation(p4[:], pE[:], EXP, bias=BIASC[:])
                state[("p4", j)] = p4
                # free-dim transpose of this chunk's k rows into K2T (DVE;
                # K2[:, csl] was written by gpsimd in row_head(j))
                csl = slice(j * 512, (j + 1) * 512)
                nc.vector.tensor_copy(
                    k2t_xm[:, :, j * 4:(j + 1) * 4],
                    k2_as_xy[:, :, j * 4:(j + 1) * 4])

            def row_tail(j):
                idx = j % 6
                p4 = state.pop(("p4", j))
                o16 = o16rp.tile([128, 4 * VBW], BF16, tag="o16r")
                for half in range(2):
                    pO = psOp.tile([128, 1024], F32, tag="psO")
                    for q2 in range(2):
                        yy = half * 2 + q2
                        nc.tensor.matmul(
                            pO[:, q2 * 512:q2 * 512 + VBW],
                            p4[:, yy * 128:(yy + 1) * 128],
                            VB[:, (idx * 4 + yy) * VBW:
                               (idx * 4 + yy + 1) * VBW],
                            start=True, stop=True)
                    for q2 in range(2):
                        yy = half * 2 + q2
                        src = pO[:, q2 * 512:q2 * 512 + VBW]
                        dst = o16[:, yy * VBW:(yy + 1) * VBW]
                        if q2 == 0:
                            nc.scalar.activation(dst, src, COPY)
                        else:
                            nc.vector.tensor_copy(dst, src)
                # orow chunk j: dst [x, 4 y, 256] contiguous per partition
                nc.sync.dma_start(
                    orow[:, j * 4:(j + 1) * 4, :],
                    o16[:].rearrange("p (t c) -> p t c", c=VBW))
                # stage VB slots to DRAM x-major: dst [x, 4 y, c] matches
                # the VB source order (x-part, yy, c); 512B descriptors
                nc.sync.dma_start(
                    VSCR[:, j * 4:(j + 1) * 4, :],
                    vb_view[:, idx * 4:idx * 4 + 4, 0:256])
                # pre-gather the first ring pairs for the column pass in
                # y-slices as soon as each VSCR y-region is complete
                # (y<64 after chunk 15, y 64..95 after chunk 23), riding
                # the row window's DMA headroom instead of stalling the
                # column-pass spin-up
                if j in (17, 19, 21, 23):
                    p = (j - 17) // 2
                    s = (2 * p) % NRING
                    nc.sync.dma_start(
                        vtb_view[0:64, s * 4:s * 4 + 8, 0:256],
                        VSCR[p * 8:(p + 1) * 8, 0:64, :].rearrange(
                            "t j c -> j t c"))


            for i in range(NCH + 2):
                if i < NCH:
                    if i + 2 < NCH:
                        load_x(i + 2)
                    row_head(i)
                if 1 <= i < NCH + 1:
                    row_mid(i - 1)
                if i >= 2:
                    row_tail(i - 2)

            # =================== column pass ===================
            # Mirrors the row pass exactly with x <-> y swapped; all
            # operands contiguous (QT/K2T x-major, VTB gathered x-major).
            cstate = {}

            def vtb_fetch_pair(p):
                # gather the 8 column slots of half-group pair p = (2p, 2p+1)
                # in ONE DMA: the sync engine's ~650ns per-DMA descriptor
                # generation is the column pass's scarcest resource
                s = (2 * p) % (NRING)
                nc.sync.dma_start(
                    vtb_view[:, s * 4:s * 4 + 8, 0:256],
                    VSCR[p * 8:(p + 1) * 8, :, :].rearrange(
                        "t j c -> j t c"))

            def col_mid(i):
                # alternate pE between the psbig and psv pools (psv is idle
                # in the column pass): 4 physical banks deep, so E(i+1)
                # depends on exp(i-1) instead of exp(i) -- keeps the scalar
                # exp latency off the tensor engine's critical loop
                if i % 2 == 0:
                    pE = psbigp.tile([128, 512], F32, tag="psbig")
                else:
                    pE = psvp.tile([128, 512], F32, tag="psv")
                for xx in range(4):
                    x = i * 4 + xx
                    xsl = slice(x * 128, (x + 1) * 128)
                    nc.tensor.matmul(pE[:, xx * 128:(xx + 1) * 128],
                                     K2T[:, xsl], QT[:, xsl],
                                     start=(xx == 0), stop=False)
                nc.tensor.matmul(pE[:], NEGID[:], ID4[:],
                                 start=False, stop=True)
                p4 = p4cp.tile([128, 512], BF16, tag="p4c")
                nc.scalar.activation(p4[:], pE[:], EXP, bias=BIASC[:])
                cstate[i] = p4

            def col_tail(i):
                p4 = cstate.pop(i)
                if i % 2 == 0:
                    cstate["o16"] = o16cp.tile([128, 8 * VBW], BF16,
                                               tag="o16c", name="o16pair")
                o16 = cstate["o16"]
                for h2 in range(2):
                    pO = psOp.tile([128, 1024], F32, tag="psO")
                    for q2 in range(2):
                        xx = h2 * 2 + q2
                        slot = (i % NRING) * 4 + xx
                        nc.tensor.matmul(
                            pO[:, q2 * 512:q2 * 512 + VBW],
                            p4[:, xx * 128:(xx + 1) * 128],
                            VTB[:, slot * VBW:(slot + 1) * VBW],
                            start=True, stop=True)
                    # one batched pair-evacuation per psO tile; keep the
                    # scalar queue nearly exp-only so exp dispatch never
                    # queues behind evac waits: DVE takes pair 0, pair 1
                    # alternates scalar/DVE by iteration parity
                    src = pO[:].rearrange("p (s w) -> p s w", w=512)[
                        :, :, 0:VBW]
                    dst = o16[:].rearrange("p (s w) -> p s w", w=VBW)[
                        :, (i % 2) * 4 + h2 * 2:(i % 2) * 4 + h2 * 2 + 2, :]
                    if h2 == 1 and i % 2 == 0:
                        nc.scalar.activation(dst, src, COPY)
                    else:
                        nc.vector.tensor_copy(dst, src)
                if i % 2 == 1:
                    # one 8-column output DMA per half-group pair
                    nc.sync.dma_start(
                        ocol[:, (i - 1) * 4:(i + 1) * 4, :],
                        o16[:].rearrange("p (t c) -> p t c", c=VBW))
                    # refill this ring pair for pair (i-1)/2 + NRING/2 now
                    # that its last reader (the agg matmuls above) has been
                    # issued -- any earlier would order those reads after
                    # the refill and hand them future data
                    p = (i - 1) // 2
                    if p + NRING // 2 < NHG // 2:
                        vtb_fetch_pair(p + NRING // 2)

            for p in range(NRING // 2):
                # top halves were pre-gathered during the row pass; only
                # the bottoms (y >= 64, complete after the last row chunk)
                # are fetched here
                s = (2 * p) % NRING
                nc.sync.dma_start(
                    vtb_view[64:128, s * 4:s * 4 + 8, 0:256],
                    VSCR[p * 8:(p + 1) * 8, 64:128, :].rearrange(
                        "t j c -> j t c"))
            for i in range(NHG + 2):
                if i < NHG:
                    col_mid(i)
                if i >= 2:
                    col_tail(i - 2)


    nc.compile()
    return nc


def _get_nc(with_bias):
    key = bool(with_bias)
    if key not in _CACHE:
        _CACHE[key] = _build(key)
    return _CACHE[key]


def kernel(x, Wq, bq, Wk, bk, Wv, bv, _trace=False, _raw=False):
    x = np.asarray(x, np.float32)
    Wq = np.asarray(Wq, np.float32)
    Wk = np.asarray(Wk, np.float32)
    Wv = np.asarray(Wv, np.float32)
    bq = np.asarray(bq, np.float32)
    bk = np.asarray(bk, np.float32)
    bv = np.asarray(bv, np.float32)

    with_bias = bool(np.any(bq) or np.any(bk) or np.any(bv))
    nc = _get_nc(with_bias)

    negid_a = (-60000.0 * np.eye(128)).astype(np.float16)
    id4_a = np.tile(np.eye(128), (1, 4)).astype(np.float16)
    wqk_full = np.concatenate([Wq.T, Wk.T], axis=1)       # [C, 128]
    if with_bias:
        bias_qk = np.concatenate([bq, bk])[None, :]
        wqk_full = np.concatenate(
            [wqk_full, bias_qk, np.zeros_like(bias_qk)], axis=0)
    wqk_full = wqk_full.astype(np.float16)

    in_maps = []
    for core in range(N_CORES):
        b, h = core // 2, core % 2
        # xin[ch, p, k*512+w] = x[b, 128k+p, ch*512+w]
        xb = np.ascontiguousarray(
            x[b].reshape(4, 128, NCH, 512).transpose(2, 1, 0, 3)
            .reshape(NCH, 128, 2048)).astype(np.float16)
        wvh = Wv[h * CV:(h + 1) * CV, :].T                # [C, CV]
        if with_bias:
            bvh = bv[h * CV:(h + 1) * CV][None, :]
            wvh = np.concatenate([wvh, bvh, np.zeros_like(bvh)], axis=0)
        m = {
            "xin": xb, "wqk": wqk_full,
            "wv": wvh.astype(np.float16),
            "negid": negid_a, "id4": id4_a,
        }
        if with_bias:
            ob = np.zeros((NCH, 2, 2048), np.float32)
            ob[:, 0, :] = 1.0
            m["xbias"] = ob.astype(np.float16)
        in_maps.append(m)

    res = run_bass_kernel_spmd(nc, in_maps, list(range(N_CORES)),
                               trace=bool(_trace))
    if _raw:
        return res

    out = np.empty((B, C, H, W), np.float32)
    for core in range(N_CORES):
        b, h = core // 2, core % 2
        r = res.results[core]
        o_r3 = r["orow"].astype(np.float32)    # [x, y, 258] unnormalized
        o_c3 = r["ocol"].astype(np.float32)    # [y, x, 258] unnormalized
        g = 1.0 / (o_r3[:, :, 256].T + o_c3[:, :, 256])        # [y, x]
        comb = (o_r3[:, :, 0:256].transpose(1, 0, 2)
                + o_c3[:, :, 0:256]) * g[:, :, None]           # [y, x, c]
        out[b, h * CV:(h + 1) * CV] = comb.transpose(2, 0, 1)

    if _trace:
        return out, res
    return out


# revision 19
# speedup vs baseline: 1.1253x; 1.1241x over previous
"""Criss-cross (CCNet) attention kernel for Trainium2, 8 NeuronCores.

Sharding: core c in 0..7 -> batch b = c//2, value-channel half h = c%2.
Each core computes, for its (b, h), the full joint row+column softmax
attention over 256 of the 512 value/output channels.

Design (v3, contiguous column pass):
  - All matmuls are fp16 (1 cyc/row at any moving size, vs fp32r's 4x
    penalty under 256), with fp32 PSUM accumulation.
  - Energies E = k.q are shifted by a global constant DELTA before exp;
    P = exp(E-DELTA) is stored in bf16 (wide exponent range covers the
    ~33-nat spread of per-row maxima; fp16 cannot).  The shift cancels
    exactly in the final combine since both passes share it.
  - Outputs are UN-normalized:  orow = P^T V  in bf16 (wide exponent
    range), plus the row/col sums S (fp32).  The host computes
    out = (o_row + o_col^T) / (S_row + S_col), which equals the exact
    softmax combine; no on-device division or scaling at all.
  - V is staged through DRAM x-major as vscr[x][j][c] (one 512B-
    descriptor write per row chunk, one strided 512B-descriptor gather
    per column chunk); the partition transpose rides the DRAM
    addressing, never a slow single-partition SBUF DMA.
  - The column pass operands are x-major SBUF copies QT/K2T written
    during the row pass (scalar evacuates QT from PSUM with a strided
    AP; DVE free-dim-transposes K2 into K2T), so every column-pass
    matmul streams contiguous SBUF exactly like the row pass.  This
    keeps the PE at its full 2.4 GHz p-state instead of the 1.2 GHz
    mid state it falls to after every stall.
  - Deep software pipeline in both passes: iteration i runs
    projections(i), energies(i-1), aggregation+evac(i-2), so the
    tensor queue never drains.
"""

import numpy as np

import concourse.tile as tile
from concourse import bacc, mybir
from concourse.bass_utils import run_bass_kernel_spmd

B, C, H, W = 4, 512, 128, 128
CQK = C // 8          # 64
CV = C // 2           # 256 v channels per core
HW = H * W
N_CORES = 8
NCH = 32              # row chunks (4 rows each)
NHG = 32              # col half-groups (4 cols each)
NRING = 8             # VTB ring size in half-groups (32 slots)
DELTA = 41.0          # exp shift (max energy on this data ~50.7)
VBW = 258             # v slot width: 256 channels + 2 ones columns

F32 = mybir.dt.float32
F16 = mybir.dt.float16
BF16 = mybir.dt.bfloat16
EXP = mybir.ActivationFunctionType.Exp
COPY = mybir.ActivationFunctionType.Copy

_CACHE = {}


def _build(with_bias):
    nc = bacc.Bacc("TRN2", target_bir_lowering=False, debug=False,
                   num_devices=N_CORES)
    nck = 5 if with_bias else 4    # contraction chunks (last = 2 bias rows)

    xin = nc.dram_tensor("xin", [NCH, 128, 2048], F16,
                         kind="ExternalInput").ap()
    xbias = nc.dram_tensor("xbias", [NCH, 2, 2048], F16,
                           kind="ExternalInput").ap() if with_bias else None
    wqk = nc.dram_tensor("wqk", [C + (2 if with_bias else 0), 128], F16,
                         kind="ExternalInput").ap()
    wv = nc.dram_tensor("wv", [C + (2 if with_bias else 0), CV], F16,
                        kind="ExternalInput").ap()
    negid = nc.dram_tensor("negid", [128, 128], F16,
                           kind="ExternalInput").ap()
    id4 = nc.dram_tensor("id4", [128, 512], F16, kind="ExternalInput").ap()

    # orow laid out [x, y, 258]; ocol laid out [y, x, 258]; channel 256 of
    # each carries the (unnormalized) softmax partition sum in bf16
    orow = nc.dram_tensor("orow", [W, H, VBW], BF16,
                          kind="ExternalOutput").ap()
    ocol = nc.dram_tensor("ocol", [H, W, VBW], BF16,
                          kind="ExternalOutput").ap()


    with tile.TileContext(nc) as tc:
        with (
            tc.tile_pool(name="cst", bufs=1) as cst,
            tc.tile_pool(name="dram", bufs=1, space="DRAM") as dramp,
            tc.tile_pool(name="xs", bufs=3) as xsp,
            tc.tile_pool(name="p4", bufs=3) as p4p,
            tc.tile_pool(name="p4c", bufs=6) as p4cp,
            tc.tile_pool(name="o16r", bufs=2) as o16rp,
            tc.tile_pool(name="o16c", bufs=4) as o16cp,
            tc.tile_pool(name="psbig", bufs=2, space="PSUM") as psbigp,
            tc.tile_pool(name="psv", bufs=2, space="PSUM") as psvp,
            tc.tile_pool(name="psO", bufs=2, space="PSUM") as psOp,
        ):
            # ---- startup: critical path first (xin0, WQK), one DMA each ----
            xpre = {}
            xt0 = xsp.tile([128, 2048], F16, tag="xs", name="xpre0")
            nc.sync.dma_start(xt0[:], xin[0])
            WQK = cst.tile([128, nck * 128], F16)
            nc.sync.dma_start(
                WQK[:].rearrange("p (k c) -> p k c", c=128)[:, 0:4, :],
                wqk[0:512, :].rearrange("(k p) c -> p k c", p=128))
            xt1 = xsp.tile([128, 2048], F16, tag="xs", name="xpre1")
            nc.sync.dma_start(xt1[:], xin[1])
            WV = cst.tile([128, nck * CV], F16)
            nc.sync.dma_start(
                WV[:].rearrange("p (k c) -> p k c", c=CV)[:, 0:4, :],
                wv[0:512, :].rearrange("(k p) c -> p k c", p=128))
            if with_bias:
                nc.sync.dma_start(WQK[0:2, 4 * 128:5 * 128], wqk[512:514, :])
                nc.sync.dma_start(WV[0:2, 4 * CV:5 * CV], wv[512:514, :])
                for ch0 in range(2):
                    xbt = xsp.tile([2, 2048], F16, tag="xb",
                                   name=f"xbpre{ch0}")
                    nc.sync.dma_start(xbt[:], xbias[ch0])
                    xpre[ch0] = ((xt0, xt1)[ch0], xbt)
            else:
                xpre[0] = (xt0, None)
                xpre[1] = (xt1, None)
            NEGID = cst.tile([128, 128], F16)
            nc.sync.dma_start(NEGID[:], negid[:])
            ID4 = cst.tile([128, 512], F16)
            nc.sync.dma_start(ID4[:], id4[:])

            QK = cst.tile([128, HW], F16)     # y-major [c, (y, x)]
            K2 = cst.tile([64, HW], F16)      # k y-major on partitions 0-63
            QT = cst.tile([64, HW], F16)      # q x-major [c, (x, y)]
            K2T = cst.tile([64, HW], F16)     # k x-major [c, (x, y)]
            VTB = cst.tile([128, NRING * 4 * VBW], BF16)
            VB = cst.tile([128, 24 * VBW], BF16)
            BIASC = cst.tile([128, 1], F32)
            nc.vector.memset(BIASC[:], -DELTA)
            # v staging scratch as a DRAM-pool tile: the tile framework
            # orders the chunk-write DMAs against the gather DMAs (raw
            # dram_tensor accesses get NO DMA-vs-DMA ordering at all)
            VSCR = dramp.tile([W, H, CV], BF16)
            vtb_view = VTB[:].rearrange("p (s w) -> p s w", w=VBW)
            vb_view = VB[:].rearrange("p (s w) -> p s w", w=VBW)
            nc.vector.memset(vtb_view[:, :, 256:258], 1.0)
            nc.vector.memset(vb_view[:, :, 256:258], 1.0)

            # x-major views [c, x, y] of the x-major tiles
            qt_xm = QT[:].rearrange("c (x y) -> c x y", y=128)
            k2t_xm = K2T[:].rearrange("c (x y) -> c x y", y=128)
            # y-major [c, x, y] views of the row-major tiles (x stride 1,
            # y stride 128) used as transpose sources
            k2_as_xy = K2[:].rearrange("c (y x) -> c x y", x=128)

            # =================== row pass ===================
            state = {}
            state[("xs", 0)] = xpre[0]
            state[("xs", 1)] = xpre[1]

            def load_x(ch):
                xt = xsp.tile([128, 2048], F16, tag="xs")
                nc.sync.dma_start(xt[:], xin[ch])
                xbt = None
                if with_bias:
                    xbt = xsp.tile([2, 2048], F16, tag="xb")
                    nc.sync.dma_start(xbt[:], xbias[ch])
                state[("xs", ch)] = (xt, xbt)

            def mm_in(xs, k, sl):
                xt, xbt = xs
                if k < 4:
                    return xt[:, k * 512:k * 512 + 512][:, sl]
                return xbt[:, sl]

            def row_head(i):
                xs = state.pop(("xs", i))
                csl = slice(i * 512, (i + 1) * 512)
                pqk = psbigp.tile([128, 512], F32, tag="psbig")
                for k in range(nck):
                    rows = 128 if k < 4 else 2
                    nc.tensor.matmul(pqk[:],
                                     WQK[0:rows, k * 128:(k + 1) * 128],
                                     mm_in(xs, k, slice(0, 512)),
                                     start=(k == 0), stop=(k == nck - 1))
                nc.scalar.activation(QK[:, csl], pqk[:], COPY)
                # second evac: q rows into the x-major QT with a strided
                # destination AP (dst (x, y), src (y, x))
                nc.scalar.activation(
                    qt_xm[:, :, i * 4:(i + 1) * 4],
                    pqk[0:64, :].rearrange("c (y x) -> c x y", x=128),
                    COPY)
                nc.gpsimd.tensor_copy(K2[:, csl], QK[64:128, csl])
                idx = i % 6
                for hv in range(2):       # two [128,512] pv tiles: yy pairs
                    pv = psvp.tile([128, 512], F32, tag="psv")
                    for q2 in range(2):
                        yy = hv * 2 + q2
                        xsl = slice(yy * 128, (yy + 1) * 128)
                        for k in range(nck):
                            rows = 128 if k < 4 else 2
                            nc.tensor.matmul(
                                pv[:, q2 * 256:(q2 + 1) * 256],
                                mm_in(xs, k, xsl),
                                WV[0:rows, k * CV:(k + 1) * CV],
                                start=(k == 0), stop=(k == nck - 1))
                    # evac both yy slots in one strided DVE copy
                    nc.vector.tensor_copy(
                        vb_view[:, idx * 4 + hv * 2:idx * 4 + hv * 2 + 2,
                                0:256],
                        pv[:].rearrange("p (s w) -> p s w", w=256))

            def row_mid(j):
                pE = psbigp.tile([128, 512], F32, tag="psbig")
                for yy in range(4):
                    y = j * 4 + yy
                    ysl = slice(y * 128, (y + 1) * 128)
                    nc.tensor.matmul(pE[:, yy * 128:(yy + 1) * 128],
                                     K2[:, ysl], QK[0:64, ysl],
                                     start=True, stop=True)
                p4 = p4p.tile([128, 512], BF16, tag="p4")
                nc.scalar.activation(p4[:], pE[:], EXP, bias=BIASC[:])
                state[("p4", j)] = p4
                # free-dim transpose of this chunk's k rows into K2T (DVE;
                # K2[:, csl] was written by gpsimd in row_head(j))
                csl = slice(j * 512, (j + 1) * 512)
                nc.vector.tensor_copy(
                    k2t_xm[:, :, j * 4:(j + 1) * 4],
                    k2_as_xy[:, :, j * 4:(j + 1) * 4])

            def row_tail(j):
                idx = j % 6
                p4 = state.pop(("p4", j))
                o16 = o16rp.tile([128, 4 * VBW], BF16, tag="o16r")
                for half in range(2):
                    pO = psOp.tile([128, 1024], F32, tag="psO")
                    for q2 in range(2):
                        yy = half * 2 + q2
                        nc.tensor.matmul(
                            pO[:, q2 * 512:q2 * 512 + VBW],
                            p4[:, yy * 128:(yy + 1) * 128],
                            VB[:, (idx * 4 + yy) * VBW:
                               (idx * 4 + yy + 1) * VBW],
                            start=True, stop=True)
                    for q2 in range(2):
                        yy = half * 2 + q2
                        src = pO[:, q2 * 512:q2 * 512 + VBW]
                        dst = o16[:, yy * VBW:(yy + 1) * VBW]
                        if q2 == 0:
                            nc.scalar.activation(dst, src, COPY)
                        else:
                            nc.vector.tensor_copy(dst, src)
                # orow chunk j: dst [x, 4 y, 256] contiguous per partition
                nc.sync.dma_start(
                    orow[:, j * 4:(j + 1) * 4, :],
                    o16[:].rearrange("p (t c) -> p t c", c=VBW))
                # stage VB slots to DRAM x-major: dst [x, 4 y, c] matches
                # the VB source order (x-part, yy, c); 512B descriptors
                nc.sync.dma_start(
                    VSCR[:, j * 4:(j + 1) * 4, :],
                    vb_view[:, idx * 4:idx * 4 + 4, 0:256])
                # pre-gather the first ring pairs for the column pass in
                # y-slices as soon as each VSCR y-region is complete
                # (y<64 after chunk 15, y 64..95 after chunk 23), riding
                # the row window's DMA headroom instead of stalling the
                # column-pass spin-up
                if j in (17, 19, 21, 23):
                    p = (j - 17) // 2
                    s = (2 * p) % NRING
                    nc.sync.dma_start(
                        vtb_view[0:64, s * 4:s * 4 + 8, 0:256],
                        VSCR[p * 8:(p + 1) * 8, 0:64, :].rearrange(
                            "t j c -> j t c"))


            for i in range(NCH + 2):
                if i < NCH:
                    if i + 2 < NCH:
                        load_x(i + 2)
                    row_head(i)
                if 1 <= i < NCH + 1:
                    row_mid(i - 1)
                if i >= 2:
                    row_tail(i - 2)

            # =================== column pass ===================
            # Mirrors the row pass exactly with x <-> y swapped; all
            # operands contiguous (QT/K2T x-major, VTB gathered x-major).
            cstate = {}

            def vtb_fetch_pair(p):
                # gather the 8 column slots of half-group pair p = (2p, 2p+1)
                # in ONE DMA: the sync engine's ~650ns per-DMA descriptor
                # generation is the column pass's scarcest resource
                s = (2 * p) % (NRING)
                nc.sync.dma_start(
                    vtb_view[:, s * 4:s * 4 + 8, 0:256],
                    VSCR[p * 8:(p + 1) * 8, :, :].rearrange(
                        "t j c -> j t c"))

            def col_mid(i):
                # alternate pE between the psbig and psv pools (psv is idle
                # in the column pass): 4 physical banks deep, so E(i+1)
                # depends on exp(i-1) instead of exp(i) -- keeps the scalar
                # exp latency off the tensor engine's critical loop
                if i % 2 == 0:
                    pE = psbigp.tile([128, 512], F32, tag="psbig")
                else:
                    pE = psvp.tile([128, 512], F32, tag="psv")
                for xx in range(4):
                    x = i * 4 + xx
                    xsl = slice(x * 128, (x + 1) * 128)
                    nc.tensor.matmul(pE[:, xx * 128:(xx + 1) * 128],
                                     K2T[:, xsl], QT[:, xsl],
                                     start=(xx == 0), stop=False)
                nc.tensor.matmul(pE[:], NEGID[:], ID4[:],
                                 start=False, stop=True)
                p4 = p4cp.tile([128, 512], BF16, tag="p4c")
                nc.scalar.activation(p4[:], pE[:], EXP, bias=BIASC[:])
                cstate[i] = p4

            def col_tail(i):
                p4 = cstate.pop(i)
                if i % 2 == 0:
                    cstate["o16"] = o16cp.tile([128, 8 * VBW], BF16,
                                               tag="o16c", name="o16pair")
                o16 = cstate["o16"]
                for h2 in range(2):
                    pO = psOp.tile([128, 1024], F32, tag="psO")
                    for q2 in range(2):
                        xx = h2 * 2 + q2
                        slot = (i % NRING) * 4 + xx
                        nc.tensor.matmul(
                            pO[:, q2 * 512:q2 * 512 + VBW],
                            p4[:, xx * 128:(xx + 1) * 128],
                            VTB[:, slot * VBW:(slot + 1) * VBW],
                            start=True, stop=True)
                    # one batched pair-evacuation per psO tile; keep the
                    # scalar queue nearly exp-only so exp dispatch never
                    # queues behind evac waits: DVE takes pair 0, pair 1
                    # alternates scalar/DVE by iteration parity
                    src = pO[:].rearrange("p (s w) -> p s w", w=512)[
                        :, :, 0:VBW]
                    dst = o16[:].rearrange("p (s w) -> p s w", w=VBW)[
                        :, (i % 2) * 4 + h2 * 2:(i % 2) * 4 + h2 * 2 + 2, :]
                    if h2 == 1 and i % 2 == 0:
                        nc.scalar.activation(dst, src, COPY)
                    else:
                        nc.vector.tensor_copy(dst, src)
                if i % 2 == 1:
                    # one 8-column output DMA per half-group pair
                    nc.sync.dma_start(
                        ocol[:, (i - 1) * 4:(i + 1) * 4, :],
                        o16[:].rearrange("p (t c) -> p t c", c=VBW))
                    # refill this ring pair for pair (i-1)/2 + NRING/2 now
                    # that its last reader (the agg matmuls above) has been
                    # issued -- any earlier would order those reads after
                    # the refill and hand them future data
                    p = (i - 1) // 2
                    if p + NRING // 2 < NHG // 2:
                        vtb_fetch_pair(p + NRING // 2)

            for p in range(NRING // 2):
                # top halves were pre-gathered during the row pass; only
                # the bottoms (y >= 64, complete after the last row chunk)
                # are fetched here
                s = (2 * p) % NRING
                nc.sync.dma_start(
                    vtb_view[64:128, s * 4:s * 4 + 8, 0:256],
                    VSCR[p * 8:(p + 1) * 8, 64:128, :].rearrange(
                        "t j c -> j t c"))
            for i in range(NHG + 2):
                if i < NHG:
                    col_mid(i)
                if i >= 2:
                    col_tail(i - 2)


    nc.compile()
    return nc


def _get_nc(with_bias):
    key = bool(with_bias)
    if key not in _CACHE:
        _CACHE[key] = _build(key)
    return _CACHE[key]


def kernel(x, Wq, bq, Wk, bk, Wv, bv, _trace=False, _raw=False):
    x = np.asarray(x, np.float32)
    Wq = np.asarray(Wq, np.float32)
    Wk = np.asarray(Wk, np.float32)
    Wv = np.asarray(Wv, np.float32)
    bq = np.asarray(bq, np.float32)
    bk = np.asarray(bk, np.float32)
    bv = np.asarray(bv, np.float32)

    with_bias = bool(np.any(bq) or np.any(bk) or np.any(bv))
    nc = _get_nc(with_bias)

    negid_a = (-60000.0 * np.eye(128)).astype(np.float16)
    id4_a = np.tile(np.eye(128), (1, 4)).astype(np.float16)
    wqk_full = np.concatenate([Wq.T, Wk.T], axis=1)       # [C, 128]
    if with_bias:
        bias_qk = np.concatenate([bq, bk])[None, :]
        wqk_full = np.concatenate(
            [wqk_full, bias_qk, np.zeros_like(bias_qk)], axis=0)
    wqk_full = wqk_full.astype(np.float16)

    in_maps = []
    for core in range(N_CORES):
        b, h = core // 2, core % 2
        # xin[ch, p, k*512+w] = x[b, 128k+p, ch*512+w]
        xb = np.ascontiguousarray(
            x[b].reshape(4, 128, NCH, 512).transpose(2, 1, 0, 3)
            .reshape(NCH, 128, 2048)).astype(np.float16)
        wvh = Wv[h * CV:(h + 1) * CV, :].T                # [C, CV]
        if with_bias:
            bvh = bv[h * CV:(h + 1) * CV][None, :]
            wvh = np.concatenate([wvh, bvh, np.zeros_like(bvh)], axis=0)
        m = {
            "xin": xb, "wqk": wqk_full,
            "wv": wvh.astype(np.float16),
            "negid": negid_a, "id4": id4_a,
        }
        if with_bias:
            ob = np.zeros((NCH, 2, 2048), np.float32)
            ob[:, 0, :] = 1.0
            m["xbias"] = ob.astype(np.float16)
        in_maps.append(m)

    res = run_bass_kernel_spmd(nc, in_maps, list(range(N_CORES)),
                               trace=bool(_trace))
    if _raw:
        return res

    out = np.empty((B, C, H, W), np.float32)
    for core in range(N_CORES):
        b, h = core // 2, core % 2
        r = res.results[core]
        o_r3 = r["orow"].astype(np.float32)    # [x, y, 258] unnormalized
        o_c3 = r["ocol"].astype(np.float32)    # [y, x, 258] unnormalized
        g = 1.0 / (o_r3[:, :, 256].T + o_c3[:, :, 256])        # [y, x]
        comb = (o_r3[:, :, 0:256].transpose(1, 0, 2)
                + o_c3[:, :, 0:256]) * g[:, :, None]           # [y, x, c]
        out[b, h * CV:(h + 1) * CV] = comb.transpose(2, 0, 1)

    if _trace:
        return out, res
    return out


# revision 21
# speedup vs baseline: 1.1415x; 1.0144x over previous
"""Criss-cross (CCNet) attention kernel for Trainium2, 8 NeuronCores.

Sharding: core c in 0..7 -> batch b = c//2, value-channel half h = c%2.
Each core computes, for its (b, h), the full joint row+column softmax
attention over 256 of the 512 value/output channels.

Design (v3, contiguous column pass):
  - All matmuls are fp16 (1 cyc/row at any moving size, vs fp32r's 4x
    penalty under 256), with fp32 PSUM accumulation.
  - Energies E = k.q are shifted by a global constant DELTA before exp;
    P = exp(E-DELTA) is stored in bf16 (wide exponent range covers the
    ~33-nat spread of per-row maxima; fp16 cannot).  The shift cancels
    exactly in the final combine since both passes share it.
  - Outputs are UN-normalized:  orow = P^T V  in bf16 (wide exponent
    range), plus the row/col sums S (fp32).  The host computes
    out = (o_row + o_col^T) / (S_row + S_col), which equals the exact
    softmax combine; no on-device division or scaling at all.
  - V is staged through DRAM x-major as vscr[x][j][c] (one 512B-
    descriptor write per row chunk, one strided 512B-descriptor gather
    per column chunk); the partition transpose rides the DRAM
    addressing, never a slow single-partition SBUF DMA.
  - The column pass operands are x-major SBUF copies QT/K2T written
    during the row pass (scalar evacuates QT from PSUM with a strided
    AP; DVE free-dim-transposes K2 into K2T), so every column-pass
    matmul streams contiguous SBUF exactly like the row pass.  This
    keeps the PE at its full 2.4 GHz p-state instead of the 1.2 GHz
    mid state it falls to after every stall.
  - Deep software pipeline in both passes: iteration i runs
    projections(i), energies(i-1), aggregation+evac(i-2), so the
    tensor queue never drains.
"""

import numpy as np

import concourse.tile as tile
from concourse import bacc, mybir
from concourse.bass_utils import run_bass_kernel_spmd

B, C, H, W = 4, 512, 128, 128
CQK = C // 8          # 64
CV = C // 2           # 256 v channels per core
HW = H * W
N_CORES = 8
NCH = 32              # row chunks (4 rows each)
NHG = 32              # col half-groups (4 cols each)
NRING = 8             # VTB ring size in half-groups (32 slots)
DELTA = 41.0          # exp shift (max energy on this data ~50.7)
VBW = 258             # v slot width: 256 channels + 2 ones columns

F32 = mybir.dt.float32
F16 = mybir.dt.float16
BF16 = mybir.dt.bfloat16
EXP = mybir.ActivationFunctionType.Exp
COPY = mybir.ActivationFunctionType.Copy

_CACHE = {}


def _build(with_bias):
    nc = bacc.Bacc("TRN2", target_bir_lowering=False, debug=False,
                   num_devices=N_CORES)
    nck = 5 if with_bias else 4    # contraction chunks (last = 2 bias rows)

    xin = nc.dram_tensor("xin", [NCH, 128, 2048], F16,
                         kind="ExternalInput").ap()
    xbias = nc.dram_tensor("xbias", [NCH, 2, 2048], F16,
                           kind="ExternalInput").ap() if with_bias else None
    wqk = nc.dram_tensor("wqk", [C + (2 if with_bias else 0), 128], F16,
                         kind="ExternalInput").ap()
    wv = nc.dram_tensor("wv", [C + (2 if with_bias else 0), CV], F16,
                        kind="ExternalInput").ap()
    negid = nc.dram_tensor("negid", [128, 128], F16,
                           kind="ExternalInput").ap()
    id4 = nc.dram_tensor("id4", [128, 512], F16, kind="ExternalInput").ap()

    # orow laid out [x, y, 258]; ocol laid out [y, x, 258]; channel 256 of
    # each carries the (unnormalized) softmax partition sum in bf16
    orow = nc.dram_tensor("orow", [W, H, VBW], BF16,
                          kind="ExternalOutput").ap()
    ocol = nc.dram_tensor("ocol", [H, W, VBW], BF16,
                          kind="ExternalOutput").ap()


    with tile.TileContext(nc) as tc:
        with (
            tc.tile_pool(name="cst", bufs=1) as cst,
            tc.tile_pool(name="dram", bufs=1, space="DRAM") as dramp,
            tc.tile_pool(name="xs", bufs=3) as xsp,
            tc.tile_pool(name="p4", bufs=3) as p4p,
            tc.tile_pool(name="p4c", bufs=6) as p4cp,
            tc.tile_pool(name="o16r", bufs=2) as o16rp,
            tc.tile_pool(name="o16c", bufs=4) as o16cp,
            tc.tile_pool(name="psbig", bufs=2, space="PSUM") as psbigp,
            tc.tile_pool(name="psv", bufs=2, space="PSUM") as psvp,
            tc.tile_pool(name="psO", bufs=2, space="PSUM") as psOp,
        ):
            # ---- startup: critical path first (xin0, WQK), one DMA each ----
            xpre = {}
            xt0 = xsp.tile([128, 2048], F16, tag="xs", name="xpre0")
            nc.sync.dma_start(xt0[:], xin[0])
            WQK = cst.tile([128, nck * 128], F16)
            nc.sync.dma_start(
                WQK[:].rearrange("p (k c) -> p k c", c=128)[:, 0:4, :],
                wqk[0:512, :].rearrange("(k p) c -> p k c", p=128))
            xt1 = xsp.tile([128, 2048], F16, tag="xs", name="xpre1")
            nc.sync.dma_start(xt1[:], xin[1])
            WV = cst.tile([128, nck * CV], F16)
            nc.sync.dma_start(
                WV[:].rearrange("p (k c) -> p k c", c=CV)[:, 0:4, :],
                wv[0:512, :].rearrange("(k p) c -> p k c", p=128))
            if with_bias:
                nc.sync.dma_start(WQK[0:2, 4 * 128:5 * 128], wqk[512:514, :])
                nc.sync.dma_start(WV[0:2, 4 * CV:5 * CV], wv[512:514, :])
                for ch0 in range(2):
                    xbt = xsp.tile([2, 2048], F16, tag="xb",
                                   name=f"xbpre{ch0}")
                    nc.sync.dma_start(xbt[:], xbias[ch0])
                    xpre[ch0] = ((xt0, xt1)[ch0], xbt)
            else:
                xpre[0] = (xt0, None)
                xpre[1] = (xt1, None)
            NEGID = cst.tile([128, 128], F16)
            nc.sync.dma_start(NEGID[:], negid[:])
            ID4 = cst.tile([128, 512], F16)
            nc.sync.dma_start(ID4[:], id4[:])

            QK = cst.tile([128, HW], F16)     # y-major [c, (y, x)]
            K2 = cst.tile([64, HW], F16)      # k y-major on partitions 0-63
            QT = cst.tile([64, HW], F16)      # q x-major [c, (x, y)]
            K2T = cst.tile([64, HW], F16)     # k x-major [c, (x, y)]
            VTB = cst.tile([128, NRING * 4 * VBW], BF16)
            VB = cst.tile([128, 24 * VBW], BF16)
            BIASC = cst.tile([128, 1], F32)
            nc.vector.memset(BIASC[:], -DELTA)
            # v staging scratch as a DRAM-pool tile: the tile framework
            # orders the chunk-write DMAs against the gather DMAs (raw
            # dram_tensor accesses get NO DMA-vs-DMA ordering at all)
            VSCR = dramp.tile([W, H, CV], BF16)
            vtb_view = VTB[:].rearrange("p (s w) -> p s w", w=VBW)
            vb_view = VB[:].rearrange("p (s w) -> p s w", w=VBW)
            nc.vector.memset(vtb_view[:, :, 256:258], 1.0)
            nc.vector.memset(vb_view[:, :, 256:258], 1.0)

            # x-major views [c, x, y] of the x-major tiles
            qt_xm = QT[:].rearrange("c (x y) -> c x y", y=128)
            k2t_xm = K2T[:].rearrange("c (x y) -> c x y", y=128)
            # y-major [c, x, y] views of the row-major tiles (x stride 1,
            # y stride 128) used as transpose sources
            k2_as_xy = K2[:].rearrange("c (y x) -> c x y", x=128)

            # =================== row pass ===================
            state = {}
            state[("xs", 0)] = xpre[0]
            state[("xs", 1)] = xpre[1]

            def load_x(ch):
                xt = xsp.tile([128, 2048], F16, tag="xs")
                nc.sync.dma_start(xt[:], xin[ch])
                xbt = None
                if with_bias:
                    xbt = xsp.tile([2, 2048], F16, tag="xb")
                    nc.sync.dma_start(xbt[:], xbias[ch])
                state[("xs", ch)] = (xt, xbt)

            def mm_in(xs, k, sl):
                xt, xbt = xs
                if k < 4:
                    return xt[:, k * 512:k * 512 + 512][:, sl]
                return xbt[:, sl]

            def row_head(i):
                xs = state.pop(("xs", i))
                csl = slice(i * 512, (i + 1) * 512)
                pqk = psbigp.tile([128, 512], F32, tag="psbig")
                for k in range(nck):
                    rows = 128 if k < 4 else 2
                    nc.tensor.matmul(pqk[:],
                                     WQK[0:rows, k * 128:(k + 1) * 128],
                                     mm_in(xs, k, slice(0, 512)),
                                     start=(k == 0), stop=(k == nck - 1))
                nc.scalar.activation(QK[:, csl], pqk[:], COPY)
                # second evac: q rows into the x-major QT with a strided
                # destination AP (dst (x, y), src (y, x))
                nc.scalar.activation(
                    qt_xm[:, :, i * 4:(i + 1) * 4],
                    pqk[0:64, :].rearrange("c (y x) -> c x y", x=128),
                    COPY)
                nc.gpsimd.tensor_copy(K2[:, csl], QK[64:128, csl])
                idx = i % 6
                for hv in range(2):       # two [128,512] pv tiles: yy pairs
                    pv = psvp.tile([128, 512], F32, tag="psv")
                    for q2 in range(2):
                        yy = hv * 2 + q2
                        xsl = slice(yy * 128, (yy + 1) * 128)
                        for k in range(nck):
                            rows = 128 if k < 4 else 2
                            nc.tensor.matmul(
                                pv[:, q2 * 256:(q2 + 1) * 256],
                                mm_in(xs, k, xsl),
                                WV[0:rows, k * CV:(k + 1) * CV],
                                start=(k == 0), stop=(k == nck - 1))
                    # evac both yy slots in one strided DVE copy
                    nc.vector.tensor_copy(
                        vb_view[:, idx * 4 + hv * 2:idx * 4 + hv * 2 + 2,
                                0:256],
                        pv[:].rearrange("p (s w) -> p s w", w=256))

            def row_mid(j):
                pE = psbigp.tile([128, 512], F32, tag="psbig")
                for yy in range(4):
                    y = j * 4 + yy
                    ysl = slice(y * 128, (y + 1) * 128)
                    nc.tensor.matmul(pE[:, yy * 128:(yy + 1) * 128],
                                     K2[:, ysl], QK[0:64, ysl],
                                     start=True, stop=True)
                p4 = p4p.tile([128, 512], BF16, tag="p4")
                nc.scalar.activation(p4[:], pE[:], EXP, bias=BIASC[:])
                state[("p4", j)] = p4
                # free-dim transpose of this chunk's k rows into K2T (DVE;
                # K2[:, csl] was written by gpsimd in row_head(j))
                csl = slice(j * 512, (j + 1) * 512)
                nc.vector.tensor_copy(
                    k2t_xm[:, :, j * 4:(j + 1) * 4],
                    k2_as_xy[:, :, j * 4:(j + 1) * 4])

            def row_tail(j):
                idx = j % 6
                p4 = state.pop(("p4", j))
                o16 = o16rp.tile([128, 4 * VBW], BF16, tag="o16r")
                for half in range(2):
                    pO = psOp.tile([128, 1024], F32, tag="psO")
                    for q2 in range(2):
                        yy = half * 2 + q2
                        nc.tensor.matmul(
                            pO[:, q2 * 512:q2 * 512 + VBW],
                            p4[:, yy * 128:(yy + 1) * 128],
                            VB[:, (idx * 4 + yy) * VBW:
                               (idx * 4 + yy + 1) * VBW],
                            start=True, stop=True)
                    for q2 in range(2):
                        yy = half * 2 + q2
                        src = pO[:, q2 * 512:q2 * 512 + VBW]
                        dst = o16[:, yy * VBW:(yy + 1) * VBW]
                        if q2 == 0:
                            nc.scalar.activation(dst, src, COPY)
                        else:
                            nc.vector.tensor_copy(dst, src)
                # orow chunk j: dst [x, 4 y, 256] contiguous per partition
                nc.sync.dma_start(
                    orow[:, j * 4:(j + 1) * 4, :],
                    o16[:].rearrange("p (t c) -> p t c", c=VBW))
                # stage VB slots to DRAM x-major: dst [x, 4 y, c] matches
                # the VB source order (x-part, yy, c); 512B descriptors
                nc.sync.dma_start(
                    VSCR[:, j * 4:(j + 1) * 4, :],
                    vb_view[:, idx * 4:idx * 4 + 4, 0:256])


            for i in range(NCH + 2):
                if i < NCH:
                    if i + 2 < NCH:
                        load_x(i + 2)
                    row_head(i)
                if 1 <= i < NCH + 1:
                    row_mid(i - 1)
                if i >= 2:
                    row_tail(i - 2)

            # =================== column pass ===================
            # Mirrors the row pass exactly with x <-> y swapped; all
            # operands contiguous (QT/K2T x-major, VTB gathered x-major).
            cstate = {}

            def vtb_fetch_pair(p):
                # gather the 8 column slots of half-group pair p = (2p, 2p+1)
                # in ONE DMA: the sync engine's ~650ns per-DMA descriptor
                # generation is the column pass's scarcest resource
                s = (2 * p) % (NRING)
                nc.sync.dma_start(
                    vtb_view[:, s * 4:s * 4 + 8, 0:256],
                    VSCR[p * 8:(p + 1) * 8, :, :].rearrange(
                        "t j c -> j t c"))

            def col_mid(i):
                # alternate pE between the psbig and psv pools (psv is idle
                # in the column pass): 4 physical banks deep, so E(i+1)
                # depends on exp(i-1) instead of exp(i) -- keeps the scalar
                # exp latency off the tensor engine's critical loop
                if i % 2 == 0:
                    pE = psbigp.tile([128, 512], F32, tag="psbig")
                else:
                    pE = psvp.tile([128, 512], F32, tag="psv")
                for xx in range(4):
                    x = i * 4 + xx
                    xsl = slice(x * 128, (x + 1) * 128)
                    nc.tensor.matmul(pE[:, xx * 128:(xx + 1) * 128],
                                     K2T[:, xsl], QT[:, xsl],
                                     start=(xx == 0), stop=False)
                nc.tensor.matmul(pE[:], NEGID[:], ID4[:],
                                 start=False, stop=True)
                p4 = p4cp.tile([128, 512], BF16, tag="p4c")
                nc.scalar.activation(p4[:], pE[:], EXP, bias=BIASC[:])
                cstate[i] = p4

            def col_tail(i):
                p4 = cstate.pop(i)
                if i % 2 == 0:
                    cstate["o16"] = o16cp.tile([128, 8 * VBW], BF16,
                                               tag="o16c", name="o16pair")
                o16 = cstate["o16"]
                for h2 in range(2):
                    pO = psOp.tile([128, 1024], F32, tag="psO")
                    for q2 in range(2):
                        xx = h2 * 2 + q2
                        slot = (i % NRING) * 4 + xx
                        nc.tensor.matmul(
                            pO[:, q2 * 512:q2 * 512 + VBW],
                            p4[:, xx * 128:(xx + 1) * 128],
                            VTB[:, slot * VBW:(slot + 1) * VBW],
                            start=True, stop=True)
                    # one batched pair-evacuation per psO tile; keep the
                    # scalar queue nearly exp-only so exp dispatch never
                    # queues behind evac waits: DVE takes pair 0, pair 1
                    # alternates scalar/DVE by iteration parity
                    src = pO[:].rearrange("p (s w) -> p s w", w=512)[
                        :, :, 0:VBW]
                    dst = o16[:].rearrange("p (s w) -> p s w", w=VBW)[
                        :, (i % 2) * 4 + h2 * 2:(i % 2) * 4 + h2 * 2 + 2, :]
                    if h2 == 1 and i % 2 == 0:
                        nc.scalar.activation(dst, src, COPY)
                    else:
                        nc.vector.tensor_copy(dst, src)
                if i % 2 == 1:
                    # one 8-column output DMA per half-group pair
                    nc.sync.dma_start(
                        ocol[:, (i - 1) * 4:(i + 1) * 4, :],
                        o16[:].rearrange("p (t c) -> p t c", c=VBW))
                    # refill this ring pair for pair (i-1)/2 + NRING/2 now
                    # that its last reader (the agg matmuls above) has been
                    # issued -- any earlier would order those reads after
                    # the refill and hand them future data
                    p = (i - 1) // 2
                    if p + NRING // 2 < NHG // 2:
                        vtb_fetch_pair(p + NRING // 2)

            for p in range(NRING // 2):
                vtb_fetch_pair(p)
            for i in range(NHG + 2):
                if i < NHG:
                    col_mid(i)
                if i >= 2:
                    col_tail(i - 2)


    nc.compile()
    return nc


def _get_nc(with_bias):
    key = bool(with_bias)
    if key not in _CACHE:
        _CACHE[key] = _build(key)
    return _CACHE[key]


def kernel(x, Wq, bq, Wk, bk, Wv, bv, _trace=False, _raw=False):
    x = np.asarray(x, np.float32)
    Wq = np.asarray(Wq, np.float32)
    Wk = np.asarray(Wk, np.float32)
    Wv = np.asarray(Wv, np.float32)
    bq = np.asarray(bq, np.float32)
    bk = np.asarray(bk, np.float32)
    bv = np.asarray(bv, np.float32)

    with_bias = bool(np.any(bq) or np.any(bk) or np.any(bv))
    nc = _get_nc(with_bias)

    negid_a = (-60000.0 * np.eye(128)).astype(np.float16)
    id4_a = np.tile(np.eye(128), (1, 4)).astype(np.float16)
    wqk_full = np.concatenate([Wq.T, Wk.T], axis=1)       # [C, 128]
    if with_bias:
        bias_qk = np.concatenate([bq, bk])[None, :]
        wqk_full = np.concatenate(
            [wqk_full, bias_qk, np.zeros_like(bias_qk)], axis=0)
    wqk_full = wqk_full.astype(np.float16)

    in_maps = []
    for core in range(N_CORES):
        b, h = core // 2, core % 2
        # xin[ch, p, k*512+w] = x[b, 128k+p, ch*512+w]
        xb = np.ascontiguousarray(
            x[b].reshape(4, 128, NCH, 512).transpose(2, 1, 0, 3)
            .reshape(NCH, 128, 2048)).astype(np.float16)
        wvh = Wv[h * CV:(h + 1) * CV, :].T                # [C, CV]
        if with_bias:
            bvh = bv[h * CV:(h + 1) * CV][None, :]
            wvh = np.concatenate([wvh, bvh, np.zeros_like(bvh)], axis=0)
        m = {
            "xin": xb, "wqk": wqk_full,
            "wv": wvh.astype(np.float16),
            "negid": negid_a, "id4": id4_a,
        }
        if with_bias:
            ob = np.zeros((NCH, 2, 2048), np.float32)
            ob[:, 0, :] = 1.0
            m["xbias"] = ob.astype(np.float16)
        in_maps.append(m)

    res = run_bass_kernel_spmd(nc, in_maps, list(range(N_CORES)),
                               trace=bool(_trace))
    if _raw:
        return res

    out = np.empty((B, C, H, W), np.float32)
    for core in range(N_CORES):
        b, h = core // 2, core % 2
        r = res.results[core]
        o_r3 = r["orow"].astype(np.float32)    # [x, y, 258] unnormalized
        o_c3 = r["ocol"].astype(np.float32)    # [y, x, 258] unnormalized
        g = 1.0 / (o_r3[:, :, 256].T + o_c3[:, :, 256])        # [y, x]
        comb = (o_r3[:, :, 0:256].transpose(1, 0, 2)
                + o_c3[:, :, 0:256]) * g[:, :, None]           # [y, x, c]
        out[b, h * CV:(h + 1) * CV] = comb.transpose(2, 0, 1)

    if _trace:
        return out, res
    return out
